# revision 1
# baseline (speedup 1.0000x reference)
"""Trainium2 Bass kernel for LocalDualDirectedMessagePassingLayer.

Strategy (8 cores, dest-sharded):
  - Each core owns 1024 destination segments (8 blocks of 128 dests).
  - dest_seg is sorted, so each dest block's edges are contiguous; host pads
    each block's edge list to BLOCK_CAP = ST_B*512 and packs, per core,
    feature-major (transposed) dense bf16 operands:
      srcT [2,128,E_CAP]  = concat(node_memory,node_features)[source_ids].T
      efts [97,E_CAP]     = concat(edge_features[edge_ids], time_encoding, ones).T
    (the ones row folds b_msg into the msg-MLP matmul).
  - Device per 512-edge super-tile: read MLP out [j,512] via 2 K-tile matmuls
    (lhsT=W_read, rhs=srcT) + ACT relu(+b_read); per 128-edge sub-tile:
    msg MLP out [e,128] (lhsT=activations, rhs=W_msg k-tiles) + DVE
    relu*scale (scale=1/cnt folded per edge, 0 for padding); one-hot S from
    iota==ldest on gpsimd; aggregation matmul accumulates msg_mean^T [j,d]
    into PSUM across the block.
  - Per block: dst-side MLP chain (agg/upd/write) -> tanh -> writeT [128,1024].
  - Host: transpose writeT, scatter rows into a copy of node_memory.
All matmul operands bf16, PSUM accumulation fp32.
"""

import sys

sys.path.insert(0, "/opt/trn_rl_repo")

import math

import ml_dtypes
import numpy as np

import concourse.bass as bass
import concourse.mybir as mybir
import concourse.tile as tile
from concourse import bacc
from concourse.bass_utils import run_bass_kernel_spmd

BF16 = ml_dtypes.bfloat16
N_CORES = 8
SUP = 512
P = 128
N_DEST = 8192
D_MEM = 128

_PROG_CACHE: dict[int, object] = {}


def _build_program(st_b: int):
    """Build the SPMD Bass program for BLOCK_CAP = st_b*512 edges per dest block."""
    nsup = 8 * st_b            # super-tiles per core
    e_cap = nsup * SUP         # padded edges per core
    nt = e_cap // P            # 128-edge sub-tiles per core

    nc = bacc.Bacc("TRN2", target_bir_lowering=False, debug=False,
                   num_devices=N_CORES)
    f32 = mybir.dt.float32
    bf16 = mybir.dt.bfloat16
    AF = mybir.ActivationFunctionType
    OP = mybir.AluOpType

    srcT = nc.dram_tensor("srcT", [2, P, e_cap], bf16, kind="ExternalInput")
    efts = nc.dram_tensor("efts", [97, e_cap], bf16, kind="ExternalInput")
    S_d = nc.dram_tensor("S_d", [P, e_cap], bf16, kind="ExternalInput")
    dstT = nc.dram_tensor("dstT", [2, P, 1024], bf16, kind="ExternalInput")
    wr = nc.dram_tensor("wr", [2, P, P], bf16, kind="ExternalInput")
    wm0 = nc.dram_tensor("wm0", [P, P], bf16, kind="ExternalInput")
    wm1 = nc.dram_tensor("wm1", [97, P], bf16, kind="ExternalInput")
    wa = nc.dram_tensor("wa", [2, P, P], bf16, kind="ExternalInput")
    wu = nc.dram_tensor("wu", [2, P, P], bf16, kind="ExternalInput")
    ww = nc.dram_tensor("ww", [P, P], bf16, kind="ExternalInput")
    br = nc.dram_tensor("br", [P, 1], f32, kind="ExternalInput")
    ba = nc.dram_tensor("ba", [P, 1], f32, kind="ExternalInput")
    bu = nc.dram_tensor("bu", [P, 1], f32, kind="ExternalInput")
    bw = nc.dram_tensor("bw", [P, 1], f32, kind="ExternalInput")
    out_d = nc.dram_tensor("writeT", [P, 1024], f32, kind="ExternalOutput")

    with tile.TileContext(nc) as tc:
        with (
            tc.tile_pool(name="const", bufs=1) as cp,
            tc.tile_pool(name="io", bufs=8) as iop,
            tc.tile_pool(name="mid", bufs=8) as midp,
            tc.tile_pool(name="rdps", bufs=2, space="PSUM") as rdps,
            tc.tile_pool(name="mgps", bufs=3, space="PSUM") as mgps,
            tc.tile_pool(name="aggps", bufs=2, space="PSUM") as aggps,
            tc.tile_pool(name="dstps", bufs=1, space="PSUM") as dstps,
        ):
            def cload(ap, shape, dtype, tag):
                t = cp.tile(shape, dtype, tag=tag)
                nc.sync.dma_start(out=t[:], in_=ap)
                return t

            dstT0 = cload(dstT[0, :, :], [P, 1024], bf16, "dstT0")
            dstT1 = cload(dstT[1, :, :], [P, 1024], bf16, "dstT1")
            wr0 = cload(wr[0, :, :], [P, P], bf16, "wr0")
            wr1 = cload(wr[1, :, :], [P, P], bf16, "wr1")
            wm0_t = cload(wm0[:, :], [P, P], bf16, "wm0")
            wm1_t = cload(wm1[:, :], [97, P], bf16, "wm1")
            wa0 = cload(wa[0, :, :], [P, P], bf16, "wa0")
            wa1 = cload(wa[1, :, :], [P, P], bf16, "wa1")
            wu0 = cload(wu[0, :, :], [P, P], bf16, "wu0")
            wu1 = cload(wu[1, :, :], [P, P], bf16, "wu1")
            ww_t = cload(ww[:, :], [P, P], bf16, "ww")
            br_t = cload(br[:, :], [P, 1], f32, "br")
            ba_t = cload(ba[:, :], [P, 1], f32, "ba")
            bu_t = cload(bu[:, :], [P, 1], f32, "bu")
            bw_t = cload(bw[:, :], [P, 1], f32, "bw")

            def dst_stage(b, agg_ps, stage, hold):
                dc = slice(b * P, (b + 1) * P)
                if stage == 0:
                    mmean = midp.tile([P, P], bf16, tag="mmean")
                    nc.vector.tensor_copy(mmean[:], agg_ps[:])
                    drp = dstps.tile([P, P], f32, tag="dst")
                    nc.tensor.matmul(drp[:], lhsT=wr0[:], rhs=dstT0[:, dc],
                                     start=True, stop=False)
                    nc.tensor.matmul(drp[:], lhsT=wr1[:], rhs=dstT1[:, dc],
                                     start=False, stop=True)
                    dstr = midp.tile([P, P], bf16, tag="dstr")
                    nc.scalar.activation(dstr[:], drp[:], AF.Relu, bias=br_t[:, :1])
                    hold.update(mmean=mmean, dstr=dstr)
                elif stage == 1:
                    agp = dstps.tile([P, P], f32, tag="dst")
                    nc.tensor.matmul(agp[:], lhsT=wa0[:], rhs=hold["dstr"][:],
                                     start=True, stop=False)
                    nc.tensor.matmul(agp[:], lhsT=wa1[:], rhs=hold["mmean"][:],
                                     start=False, stop=True)
                    aggT = midp.tile([P, P], bf16, tag="aggT")
                    nc.scalar.activation(aggT[:], agp[:], AF.Relu, bias=ba_t[:, :1])
                    hold.update(aggT=aggT)
                elif stage == 2:
                    upp = dstps.tile([P, P], f32, tag="dst")
                    nc.tensor.matmul(upp[:], lhsT=wu0[:], rhs=hold["aggT"][:],
                                     start=True, stop=False)
                    nc.tensor.matmul(upp[:], lhsT=wu1[:], rhs=hold["dstr"][:],
                                     start=False, stop=True)
                    updT = midp.tile([P, P], bf16, tag="updT")
                    nc.scalar.activation(updT[:], upp[:], AF.Relu, bias=bu_t[:, :1])
                    hold.update(updT=updT)
                else:
                    wrp = dstps.tile([P, P], f32, tag="dst")
                    nc.tensor.matmul(wrp[:], lhsT=ww_t[:], rhs=hold["updT"][:],
                                     start=True, stop=True)
                    wout = midp.tile([P, P], f32, tag="wout")
                    nc.scalar.activation(wout[:], wrp[:], AF.Tanh, bias=bw_t[:, :1])
                    nc.sync.dma_start(out=out_d[:, dc], in_=wout[:])

            pending = None
            hold = {}
            for b in range(8):
                agg_ps = aggps.tile([P, P], f32, tag="agg")
                dbl = {}
                for st in range(st_b):
                    off = (b * st_b + st) * SUP
                    if st % 2 == 0:
                        w = min(2 * SUP, (st_b - st) * SUP)
                        s0d = iop.tile([P, 2 * SUP], bf16, tag="s0")
                        nc.sync.dma_start(out=s0d[:, :w],
                                          in_=srcT[0, :, off:off + w])
                        s1d = iop.tile([P, 2 * SUP], bf16, tag="s1")
                        nc.scalar.dma_start(out=s1d[:, :w],
                                            in_=srcT[1, :, off:off + w])
                        efd = iop.tile([97, 2 * SUP], bf16, tag="ef")
                        nc.gpsimd.dma_start(out=efd[:, :w],
                                            in_=efts[:, off:off + w])
                        S4d = iop.tile([P, 2 * SUP], bf16, tag="S4")
                        nc.sync.dma_start(out=S4d[:, :w],
                                          in_=S_d[:, off:off + w])
                        dbl = dict(s0d=s0d, s1d=s1d, efd=efd, S4d=S4d)
                    half = slice((st % 2) * SUP, (st % 2) * SUP + SUP)
                    s0, s1 = dbl["s0d"][:, half], dbl["s1d"][:, half]
                    ef, S4 = dbl["efd"][:, half], dbl["S4d"][:, half]

                    rd = rdps.tile([P, SUP], f32, tag="rd")
                    nc.tensor.matmul(rd[:], lhsT=wr0[:], rhs=s0,
                                     start=True, stop=False)
                    nc.tensor.matmul(rd[:], lhsT=wr1[:], rhs=s1,
                                     start=False, stop=True)
                    srT = midp.tile([P, SUP], bf16, tag="srT")
                    nc.scalar.activation(srT[:], rd[:], AF.Relu, bias=br_t[:, :1])

                    for q in range(4):
                        qs = slice(q * P, (q + 1) * P)
                        mg = mgps.tile([P, P], f32, tag="mg")
                        nc.tensor.matmul(mg[:], lhsT=srT[:, qs], rhs=wm0_t[:],
                                         start=True, stop=False)
                        nc.tensor.matmul(mg[:], lhsT=ef[:, qs], rhs=wm1_t[:],
                                         start=False, stop=True)
                        msgs = midp.tile([P, P], bf16, tag="msgs")
                        nc.vector.tensor_scalar_max(msgs[:], mg[:], 0.0)
                        nc.tensor.matmul(agg_ps[:], lhsT=msgs[:], rhs=S4[:, qs],
                                         start=(st == 0 and q == 0),
                                         stop=(st == st_b - 1 and q == 3))

                    if pending is not None and st < 4:
                        dst_stage(pending[0], pending[1], st, hold)
                        if st == 3:
                            pending = None
                if pending is not None:
                    for stage in range(min(st_b, 4), 4):
                        dst_stage(pending[0], pending[1], stage, hold)
                pending = (b, agg_ps)
                hold = {}
            for stage in range(4):
                dst_stage(pending[0], pending[1], stage, hold)

    nc.finalize()
    return nc


def _prep_inputs(inputs):
    """Host-side shard/pack. Returns (in_maps, st_b, node_memory, node_ids)."""
    node_memory = np.ascontiguousarray(np.asarray(inputs["node_memory"], np.float32))
    node_features = np.asarray(inputs["node_features"], np.float32)
    edge_features = np.asarray(inputs["edge_features"], np.float32)
    time_encoding = np.asarray(inputs["time_encoding"], np.float32)
    node_ids = np.asarray(inputs["node_ids"]).astype(np.int64)
    source_ids = np.asarray(inputs["source_ids"]).astype(np.int64)
    edge_ids = np.asarray(inputs["edge_ids"]).astype(np.int64)
    dest_seg = np.asarray(inputs["dest_seg"]).astype(np.int64)
    W_read = np.asarray(inputs["W_read"], np.float32)
    b_read = np.asarray(inputs["b_read"], np.float32)
    W_msg = np.asarray(inputs["W_msg"], np.float32)
    b_msg = np.asarray(inputs["b_msg"], np.float32)
    W_agg = np.asarray(inputs["W_agg"], np.float32)
    b_agg = np.asarray(inputs["b_agg"], np.float32)
    W_upd = np.asarray(inputs["W_upd"], np.float32)
    b_upd = np.asarray(inputs["b_upd"], np.float32)
    W_write = np.asarray(inputs["W_write"], np.float32)
    b_write = np.asarray(inputs["b_write"], np.float32)

    n_edge = dest_seg.shape[0]

    cnt = np.bincount(dest_seg, minlength=N_DEST)
    inv_cnt = np.zeros(N_DEST, np.float32)
    nz = cnt > 0
    inv_cnt[nz] = 1.0 / cnt[nz]

    # 64 global dest blocks of 128; block B's edges are dest_seg in [B*128,(B+1)*128)
    bounds = np.searchsorted(dest_seg, np.arange(0, N_DEST + 1, P))
    per_block = np.diff(bounds)
    st_b = max(1, math.ceil(per_block.max() / SUP))
    block_cap = st_b * SUP
    e_cap = 8 * block_cap
    nt = e_cap // P

    # per-core edge selection (padded); esel indexes into the edge arrays
    esel = np.zeros((N_CORES, e_cap), np.int64)
    valid = np.zeros((N_CORES, e_cap), bool)
    for c in range(N_CORES):
        for blk in range(8):
            B = c * 8 + blk
            lo, hi = int(bounds[B]), int(bounds[B + 1])
            off = blk * block_cap
            esel[c, off:off + hi - lo] = np.arange(lo, hi)
            valid[c, off:off + hi - lo] = True
    esel_f = esel.reshape(-1)
    valid_f = valid.reshape(-1)

    nodecat = np.concatenate([node_memory, node_features], axis=1)  # [N,256]

    src_rows = nodecat[source_ids[esel_f]]
    src_rows[~valid_f] = 0.0
    srcT = np.ascontiguousarray(
        src_rows.reshape(N_CORES, e_cap, 256).transpose(0, 2, 1)
    ).astype(BF16).reshape(N_CORES, 2, P, e_cap)

    ef_rows = edge_features[edge_ids[esel_f]]
    t_rows = time_encoding[np.minimum(esel_f, n_edge - 1)]
    eft = np.concatenate(
        [ef_rows, t_rows, np.ones((len(esel_f), 1), np.float32)], axis=1)
    eft[~valid_f] = 0.0
    eft[valid_f, 96] = 1.0
    efts = np.ascontiguousarray(
        eft.reshape(N_CORES, e_cap, 97).transpose(0, 2, 1)).astype(BF16)

    scale_e = inv_cnt[dest_seg[esel_f]]
    scale_e[~valid_f] = 0.0
    ld_e = dest_seg[esel_f] % P
    ld_e[~valid_f] = 0
    S_flat = np.zeros((N_CORES * e_cap, P), np.float32)
    S_flat[np.arange(N_CORES * e_cap), ld_e] = scale_e
    S_pack = np.ascontiguousarray(
        S_flat.reshape(N_CORES, nt, P, P).transpose(0, 2, 1, 3)
        .reshape(N_CORES, P, e_cap)).astype(BF16)

    drows = nodecat[node_ids]                                  # [8192, 256]
    dstT = np.ascontiguousarray(
        drows.reshape(N_CORES, 1024, 256).transpose(0, 2, 1)
    ).astype(BF16).reshape(N_CORES, 2, P, 1024)

    wr_h = np.ascontiguousarray(W_read.reshape(2, P, P)).astype(BF16)
    wm0_h = np.ascontiguousarray(W_msg[:P]).astype(BF16)
    wm1_h = np.ascontiguousarray(
        np.concatenate([W_msg[P:], b_msg[None, :]], axis=0)).astype(BF16)
    wa_h = np.ascontiguousarray(W_agg.reshape(2, P, P)).astype(BF16)
    wu_h = np.ascontiguousarray(W_upd.reshape(2, P, P)).astype(BF16)
    ww_h = np.ascontiguousarray(W_write).astype(BF16)
    br_h = np.ascontiguousarray(b_read[:, None]).astype(np.float32)
    ba_h = np.ascontiguousarray(b_agg[:, None]).astype(np.float32)
    bu_h = np.ascontiguousarray(b_upd[:, None]).astype(np.float32)
    bw_h = np.ascontiguousarray(b_write[:, None]).astype(np.float32)

    in_maps = []
    for c in range(N_CORES):
        in_maps.append({
            "srcT": srcT[c], "efts": efts[c], "S_d": S_pack[c],
            "dstT": dstT[c],
            "wr": wr_h, "wm0": wm0_h, "wm1": wm1_h, "wa": wa_h, "wu": wu_h,
            "ww": ww_h, "br": br_h, "ba": ba_h, "bu": bu_h, "bw": bw_h,
        })
    return in_maps, st_b, node_memory, node_ids


def run(inputs, trace=False, **kw):
    in_maps, st_b, node_memory, node_ids = _prep_inputs(inputs)
    if st_b not in _PROG_CACHE:
        _PROG_CACHE[st_b] = _build_program(st_b)
    nc = _PROG_CACHE[st_b]
    res = run_bass_kernel_spmd(nc, in_maps, core_ids=list(range(N_CORES)),
                               trace=trace, **kw)
    wt = np.concatenate(
        [np.asarray(res.results[c]["writeT"], np.float32).T
         for c in range(N_CORES)], axis=0)             # [8192, 128]
    out = node_memory.copy()
    out[node_ids] = wt
    return out, res


def kernel(**inputs) -> np.ndarray:
    out, _ = run(inputs, trace=False)
    return out

